# revision 33
# baseline (speedup 1.0000x reference)
"""TRN2 Bass kernel for nn_Attention_70257075028315.

reference:
    scores = einsum('bqd,bkd->bqk', query, key)       # B=8, Nq=Nk=2048, D=512
    probs  = softmax(scores, -1)
    out    = einsum('bqk,bkd->bqd', probs, key)

Sharding: batch b -> NeuronCore b (data parallel, fully local attention).

v3 design (per-core q/k: [2048, 512] fp16 -- host casts fp32->fp16 so the
device never sees fp32 inputs; halves HBM load traffic and kills the cast):

  Load (all DMA, zero PE/DVE/ACT work):
    k16 [P, 16t, 512]  natural fp16, plain SWDGE loads (gpsimd queue)
    kT  [P, 4g, 4dc, 4t, P] via direct DRAM->SBUF XBAR dma-transposes, one
        per 4-tile group ([512,512] fp16 source). Group-major layout makes
        the S rhs slice for score-chunk c (= K group c) contiguous.
    qT  [P, 4g, 4dc, 4t, P] same, lazily per group (g0 up front).
    16 warm-up matmuls on a zero tile run during the DMA fill so the PE's
    HAM clock-gate (1.2->2.4 GHz after ~3.4us busy) is warm for S(0).

  Phase C per q-tile i (steady state, software-pipelined, lookahead=2):
    S     16 fp16 matmuls (4 dc-accum x 4 chunks) in PAIR-interleaved order
          (c0,c1 alternate, then c2,c3): consecutive MMs hit different PSUM
          banks so accumulation RMW chains never serialize (same-bank
          back-to-back accum MMs cost the full ~379ns isolated-MM latency
          instead of ~131ns streamed -- HW-measured), while chunk c0/c1
          still finish at MM#7/8 so the max pipeline starts at mid-tile.
    max   per-chunk DVE row-max emitted at each chunk's stop (DVE reduce is
          1x-mode, one PSUM operand per instruction), then negate-combine;
          pair order hides the c0/c1 maxes under S's second half.
    p     = exp(S - max) fp16, one ACT pass per chunk with fused row-sum
          accum; chunk-wise XBAR SBUF->SBUF transposes into pT as each
          exp lands (xbar_splits=4; 1 and 2 measured slower).
    PV    16 fp16 matmuls, accumulation INTERLEAVED over 2 PSUM banks
          (t%2) so the 16-deep accumulation chain becomes 2 chains and
          streams at MM rate; partials merged on DVE (copy + one
          PSUM+SBUF add -- the "fused" ACT-PSUM-mul variant measured
          1.5us/tile slower) and scaled by 1/rowsum on ACT, out-DMA on
          the Pool queue. Row-sum combine + reciprocal are emitted after
          the merge adds so the DVE queue never idles on fresh exps.
  PSUM: 6 "s" banks (ring; S(i+1) never waits on tile i's exp chain) +
  2 "pv" banks.

Measured: v2 baseline phase C 143.2us (load +74us on top); v3 phase C
118.9us, rel err 1.505e-03. Decomposition that motivated v3: S-only
31.3us/16 tiles (fp16 MMs stream ~2 cols/cycle, ~122ns per N=512 MM);
S+PV mm-only 135.6us -> v2 PV's single-bank accumulation chain ran at
the ~379ns isolated-MM latency per MM. lookahead=3, xbar_splits in
{1,2}, s_order dc/c, and ACT-side merges all measured worse.
"""

import numpy as np

import concourse.bass as bass
import concourse.tile as tile
import concourse.mybir as mybir
from concourse import bacc
from concourse.bass_utils import run_bass_kernel_spmd

FP32 = mybir.dt.float32
FP32R = mybir.dt.float32r
FP16 = mybir.dt.float16
AF = mybir.ActivationFunctionType
ALU = mybir.AluOpType

B, NQ, NK, D = 8, 2048, 2048, 512
P = 128
NKT = NK // P   # 16 kk tiles
NQT = NQ // P   # 16 q tiles
NDC = D // P    # 4 d chunks
NCH = NK // 512  # 4 score chunks of 512
NG = 4          # 4-tile load groups


def build(score_dtype=FP16, repeat_c=1, timed=False, pv_dtype=FP16,
          xbar_splits=4, lookahead=2, s_order="pair", probe=None,
          dedup_ldw=True, timed_scope="c", pv_banks=2, warmup=16,
          merge_eng="dve", pt_queues=1):
    """timed=True adds an int32 [1,1] input "reps": phase C (timed_scope="c")
    or the whole kernel (="full") re-runs in a dynamic For_i loop `reps`
    more times, so one NEFF measures its own steady-state slope."""
    nc = bacc.Bacc("TRN2", target_bir_lowering=False, debug=False)
    q_d = nc.dram_tensor("query", [NQ, D], FP16, kind="ExternalInput").ap()
    k_d = nc.dram_tensor("key", [NK, D], FP16, kind="ExternalInput").ap()
    reps_d = None
    if timed:
        reps_d = nc.dram_tensor(
            "reps", [1, 1], mybir.dt.int32, kind="ExternalInput"
        ).ap()
    out_d = nc.dram_tensor("out", [NQ, D], FP32, kind="ExternalOutput").ap()

    with tile.TileContext(nc) as tc:
        _body(tc, q_d, k_d, out_d, score_dtype, repeat_c, reps_d, pv_dtype,
              xbar_splits, lookahead, s_order, probe, timed_scope, pv_banks,
              warmup, merge_eng, pt_queues)
    nc.compile()
    if dedup_ldw:
        # NOTE: do NOT reorder instructions here: compile() lowers waits to
        # per-engine completion-count semaphores, so same-engine reordering
        # breaks wait semantics. Removing a non-updating instruction and
        # merging its waits forward is the only safe post-compile transform.
        _dedup_ldweights(nc)
    return nc


def _dedup_ldweights(nc):
    """Drop InstLdweights that reload the exact weights already in the PE
    array (pair-order S keeps lhsT constant across 2 consecutive matmuls,
    warm-up MMs across 16). Safe: the weights value is identical, and the
    removed instruction's dependency edges are merged into its matmul."""
    def wkey(ldw):
        ap = ldw.ins[0]
        return (str(ap.memref), ap.offset, str(ap.ap), str(ap.dtype),
                str(ldw.is_transpose), str(getattr(ldw, "perf_mode", None)))

    removed = {}
    for fn in nc.m.functions:
        for blk in fn.blocks:
            insl = list(blk.instructions)
            next_pe = [None] * len(insl)
            nxt = None
            for i in range(len(insl) - 1, -1, -1):
                next_pe[i] = nxt
                if str(insl[i].engine) == "EngineType.PE":
                    nxt = insl[i]
            keep, last = [], None
            for i, ins in enumerate(insl):
                if str(ins.engine) != "EngineType.PE":
                    keep.append(ins)
                    continue
                if type(ins).__name__ != "InstLdweights":
                    keep.append(ins)
                    continue
                k = wkey(ins)
                mm = next_pe[i]
                if (k == last and mm is not None
                        and type(mm).__name__ == "InstMatmult"):
                    mm.merge_dependencies_from(ins)
                    removed[ins.name] = mm.name
                    continue
                last = k
                keep.append(ins)
            if len(keep) != len(insl):
                blk.instructions = keep
    if removed:
        for fn in nc.m.functions:
            for blk in fn.blocks:
                for ins in blk.instructions:
                    ins.remap_dependency_names(removed)


def _body(tc, q_d, k_d, out_d, score_dtype, repeat_c, reps_d, pv_dtype,
          xbar_splits, lookahead, s_order, probe, timed_scope, pv_banks,
          warmup, merge_eng="dve", pt_queues=1):
    from contextlib import ExitStack

    nc = tc.nc
    q_tiles_d = q_d.rearrange("(t p) d -> t p d", p=P)
    k_tiles_d = k_d.rearrange("(t p) d -> t p d", p=P)
    out_tiles_d = out_d.rearrange("(t p) d -> t p d", p=P)

    reps_rv = None
    if reps_d is not None:
        regs = nc.alloc_registers("reps_regs")
        nc.regs_load(regs, reps_d[0:1, 0:1])
        reps_rv = nc.snap(regs, donate=True, min_val=0, max_val=64)
    with ExitStack() as ctx:
        persist = ctx.enter_context(tc.tile_pool(name="persist", bufs=1))
        work = ctx.enter_context(tc.tile_pool(name="work", bufs=4))
        small = ctx.enter_context(tc.tile_pool(name="small", bufs=4))
        ps_s = ctx.enter_context(
            tc.tile_pool(name="ps_s", bufs=8 - pv_banks, space="PSUM"))
        ps_pv = ctx.enter_context(
            tc.tile_pool(name="ps_pv", bufs=pv_banks, space="PSUM"))

        # SBUF layouts. kT/qT group-major: one DRAM->SBUF XBAR transpose of
        # a [512, 512] fp16 source fills kT[:, g] with [d%128, dc, t, p]
        # (the XBAR writes col-block j=dc to free offset j*512 + row), so
        # the S rhs for chunk c / dchunk dc is the contiguous kT[:, c, dc].
        k16 = persist.tile([P, NKT, D], pv_dtype)
        kT = persist.tile([P, NG, NDC, 4, P], score_dtype)
        qT = persist.tile([P, NG, NDC, 4, P], score_dtype)
        qT_s = lambda dc, i: qT[:, i // 4, dc, i % 4, :]

        # Warm the ACT exp table at t=0 so the first real exp doesn't pay
        # the ~1.3us table load.
        warm = persist.tile([P, 1], FP32)
        nc.vector.memset(warm[:], 0.0)
        nc.scalar.activation(warm[:], warm[:], AF.Exp, bias=warm[:])

        # Ablation-probe scaffolding (timing only; output wrong):
        #   no_max: constant exp bias (removes DVE maxes + S->exp coupling)
        #   no_pT : constant PV lhsT (removes pT xbars + exp->PV coupling);
        #           p is DMA-consumed so walrus can't dead-code the chain.
        cbias = None
        dummy_pT = None
        sink_d = None
        if probe == "no_max":
            cbias = persist.tile([P, 1], FP32)
            nc.vector.memset(cbias[:], -60.0)
        if probe == "no_pT":
            dummy_pT = persist.tile([P, NKT, P], pv_dtype)
            nc.vector.memset(dummy_pT[:], 0.001)
            sink_d = nc.dram_tensor(
                "probe_sink", [P, NCH], pv_dtype, kind="Internal").ap()

        # PE warm-up during the DMA fill: ~16 N=512 matmuls on a zero tile
        # (~3.4us) flip the HAM clock-gate to 2.4 GHz before S(0) lands.
        if warmup:
            warm16 = persist.tile([P, 512], score_dtype)
            nc.vector.memset(warm16[:], 0.0)
            wps = ps_pv.tile([P, 512], FP32, tag="pv", name="warmup_ps")
            for _ in range(warmup):
                nc.tensor.matmul(wps[:], lhsT=warm16[:, 0:P], rhs=warm16[:],
                                 start=True, stop=True)

        def emit_kT_g(g, eng):
            eng.dma_start_transpose(
                kT[:, g].rearrange("p a t r -> p a (t r)"),
                k_d[g * 512:(g + 1) * 512, :])

        def emit_qT_g(g, eng):
            eng.dma_start_transpose(
                qT[:, g].rearrange("p a t r -> p a (t r)"),
                q_d[g * 512:(g + 1) * 512, :])

        def emit_k16_g(g, eng):
            src = k_tiles_d[g * 4:(g + 1) * 4].rearrange("t p d -> p t d")
            eng.dma_start(k16[:, g * 4:(g + 1) * 4], src)

        q_groups_emitted = [False] * NG

        def emit_head():
            q_groups_emitted[:] = [False] * NG
            # Need-order: tile 0 runs chunk-order, chunk c needs kT group c.
            # sync gets g0/g2, scalar qT g0 then g1/g3; k16 on gpsimd.
            emit_kT_g(0, nc.sync)
            emit_qT_g(0, nc.scalar)
            emit_k16_g(0, nc.gpsimd)
            emit_kT_g(1, nc.scalar)
            emit_kT_g(2, nc.sync)
            emit_kT_g(3, nc.scalar)
            for g in range(1, NG):
                emit_k16_g(g, nc.gpsimd)
            q_groups_emitted[0] = True

        def ensure_q_group(i):
            g = i // 4
            if not q_groups_emitted[g]:
                emit_qT_g(g, nc.scalar)
                q_groups_emitted[g] = True

        emit_head()

        # ---- Phase C ----
        def emit_S(i, c_order=False):
            """S matmuls + fused pairwise chunk maxes.
            Pair order: (c0,c1) alternate across dc, then (c2,c3): no
            same-bank back-to-back accumulation (see module docstring),
            chunks still complete in two waves for early max/exp start."""
            # Two dual-bank tiles (c0,c1 | c2,c3): halves the DVE max
            # instruction count (2 reduces of FD=1024 vs 4 of FD=512 --
            # DVE ops carry ~120cyc PSUM overhead + pipe-drain each).
            # bufs=3 -> 6 banks, ring: S(i+1) reuses S(i)'s first pair,
            # freed by exp(i,c0/c1) before S(i+1) reaches c2/c3.
            s2 = [ps_s.tile([P, 2, 512], FP32, tag="s2", name=f"s{i}_{j}",
                            bufs=3) for j in range(2)]
            chunks = [s2[0][:, 0, :], s2[0][:, 1, :],
                      s2[1][:, 0, :], s2[1][:, 1, :]]
            m2 = small.tile([P, 2], FP32, tag="m2")
            negmax = small.tile([P, 1], FP32, tag="negmax")
            if c_order or s_order == "c":
                loop = [(dc, c) for c in range(NCH) for dc in range(NDC)]
            elif s_order == "dc":
                loop = [(dc, c) for dc in range(NDC) for c in range(NCH)]
            else:  # pair
                loop = [(dc, cp + c) for cp in (0, 2)
                        for dc in range(NDC) for c in (0, 1)]
            last_mm = None
            for dc, c in loop:
                last_mm = nc.tensor.matmul(
                    chunks[c][:],
                    lhsT=qT_s(dc, i),
                    rhs=kT[:, c, dc],
                    start=(dc == 0),
                    stop=(dc == NDC - 1),
                )
                if dc == NDC - 1 and probe != "no_max" and c in (1, 3):
                    # Pair row-max right at the pair's stop -- pair order
                    # stops c0/c1 mid-tile so max01 hides under S's 2nd half.
                    nc.vector.reduce_max(
                        m2[:, c // 2:c // 2 + 1],
                        s2[c // 2][:].rearrange("p a b -> p (a b)"),
                        axis=mybir.AxisListType.X,
                    )
            if probe == "no_max":
                return chunks, cbias, last_mm
            nc.vector.reduce_max(
                negmax[:], m2[:], axis=mybir.AxisListType.X, negate=True)
            return chunks, negmax, last_mm

        def emit_E(i, chunks, negmax):
            """exp(S - max) per chunk -> p (fp16) + fused row-sums; each
            chunk XBAR-transposed into pT as its exp lands. Row-sum
            combine + reciprocal are emitted LATER (emit_sums) so the DVE
            queue isn't blocked waiting on the last exp before running the
            PV merge adds of the tile 2 steps behind."""
            p = work.tile([P, NCH, 512], pv_dtype, tag="p")
            pT = work.tile([P, NKT, P], pv_dtype, tag="pT")
            rs4 = small.tile([P, NCH], FP32, tag="rs4")
            rowsum = small.tile([P, 1], FP32, tag="rowsum")
            rinv = small.tile([P, 1], FP32, tag="rinv")
            w = NCH // xbar_splits
            for c in range(NCH):
                nc.scalar.activation(
                    p[:, c, :], chunks[c][:], AF.Exp, bias=negmax[:],
                    accum_out=rs4[:, c:c + 1],
                )
                if probe == "no_pT":
                    continue
                if (c + 1) % w == 0:
                    s = c + 1 - w
                    xq = (nc.sync if pt_queues == 1 or i % 2 == 0
                          else nc.scalar)
                    xq.dma_start_transpose(
                        pT[:, s * 4:(c + 1) * 4, :], p[:, s:c + 1, :]
                    )
            if probe == "no_pT":
                nc.scalar.dma_start(sink_d[:, :], p[:, :, 0:1])
                return dummy_pT, (rs4, rowsum, rinv)
            return pT, (rs4, rowsum, rinv)

        def emit_sums(sums):
            rs4, rowsum, rinv = sums
            nc.vector.reduce_sum(rowsum[:], rs4[:], axis=mybir.AxisListType.X)
            nc.vector.reciprocal(rinv[:], rowsum[:])

        def emit_PV(i, pT, rinv, after=None, sums_cur=None):
            """PV with accumulation interleaved over pv_banks PSUM banks
            (t % pv_banks) so accumulation chains stream at MM rate; the
            partials are merged on DVE (+GPSIMD for the SBUF-only final
            add) and scaled by 1/rowsum on ACT."""
            nb = pv_banks
            pvs = [ps_pv.tile([P, 512], FP32, tag="pv", name=f"pv{i}_{j}")
                   for j in range(nb)]
            for t in range(NKT):
                mm = nc.tensor.matmul(
                    pvs[t % nb][:],
                    lhsT=pT[:, t, :],
                    rhs=k16[:, t, :],
                    start=(t < nb),
                    stop=(t >= NKT - nb),
                )
                if t == 0 and after is not None:
                    # Keep PV(i) behind S(i+2) on the PE queue so PV's work
                    # covers tile i+2's max->exp->xbar latency chain.
                    tile.add_dep_helper(
                        mm.ins, after.ins, False, "pv-after-next-S"
                    )
            # Merge the partials: DVE may read only one PSUM operand per
            # instruction, so copy pv0 to SBUF then add pv1 (PSUM) + copy
            # (SBUF). Scale by 1/rowsum on ACT (frees DVE for next tile's
            # maxes); row-sum combine + reciprocal are slotted here so the
            # DVE queue never waits on the freshly-emitted exps.
            # Merge + scale (A/B-measured: DVE copy + DVE add + ACT scale
            # beats the "fused" ACT-PSUM-mul variant by ~1.5us/tile).
            acc = work.tile([P, 512], FP32, tag="acc")
            merged = work.tile([P, 512], FP32, tag="merged")
            out_sb = work.tile([P, 512], FP32, tag="out_sb")
            nc.vector.tensor_copy(acc[:], pvs[0][:])
            for j in range(1, nb):
                dst = merged if j == nb - 1 else acc
                nc.vector.scalar_tensor_tensor(
                    dst[:], pvs[j][:], 0.0, acc[:], ALU.add, ALU.add)
            if sums_cur is not None:
                emit_sums(sums_cur)
            nc.scalar.mul(out_sb[:], merged[:], rinv[:])
            oq = nc.sync if i >= NQT - 1 else nc.gpsimd
            oq.dma_start(out_tiles_d[i], out_sb[:])
            return mm

        def emit_C():
            if probe == "s_only":
                for i in range(NQT):
                    ensure_q_group(i)
                    emit_S(i, c_order=(i == 0))
                return
            if probe == "mm_only":
                # Pure-PE probe: S + interleaved-bank PV with a constant
                # lhsT (no softmax coupling). Output numerically wrong.
                for i in range(NQT):
                    ensure_q_group(i)
                    emit_S(i, c_order=(i == 0))
                    nb = pv_banks
                    pvs = [ps_pv.tile([P, 512], FP32, tag="pv",
                                      name=f"mpv{i}_{j}") for j in range(nb)]
                    for t in range(NKT):
                        nc.tensor.matmul(
                            pvs[t % nb][:], lhsT=qT_s(0, i), rhs=k16[:, t, :],
                            start=(t < nb), stop=(t >= NKT - nb),
                        )
                return
            la = lookahead
            state = {}
            for j in range(la):
                ensure_q_group(j)
                s_ps, s_nm, _ = emit_S(j, c_order=(j == 0))
                pT_j, sums_j = emit_E(j, s_ps, s_nm)
                emit_sums(sums_j)
                state[j] = (pT_j, sums_j[2])
            for i in range(NQT):
                pT_i, rinv_i = state.pop(i)
                after = None
                sums_cur = None
                if i + la < NQT:
                    ensure_q_group(i + la)
                    s_ps, s_nm, after = emit_S(i + la)
                    pT_n, sums_cur = emit_E(i + la, s_ps, s_nm)
                    state[i + la] = (pT_n, sums_cur[2])
                emit_PV(i, pT_i, rinv_i, after=after, sums_cur=sums_cur)

        emit_C()
        for _ in range(repeat_c - 1):
            emit_C()

        if reps_rv is not None:
            with tc.For_i(0, reps_rv, 1):
                if timed_scope in ("full", "load"):
                    emit_head()
                if timed_scope == "load":
                    for g in range(NG):
                        ensure_q_group(g * 4)
                else:
                    emit_C()


_NC_CACHE = {}


def _get_nc(repeat_c=1):
    key = repeat_c
    if key not in _NC_CACHE:
        _NC_CACHE[key] = build(repeat_c=repeat_c)
    return _NC_CACHE[key]


def kernel(query: np.ndarray, key: np.ndarray) -> np.ndarray:
    query = np.asarray(query, dtype=np.float32)
    key = np.asarray(key, dtype=np.float32)
    assert query.shape == (B, NQ, D) and key.shape == (B, NK, D)
    q16 = query.astype(np.float16)
    k16 = key.astype(np.float16)
    nc = _get_nc()
    in_maps = [{"query": q16[b], "key": k16[b]} for b in range(B)]
    res = run_bass_kernel_spmd(nc, in_maps, list(range(B)))
    return np.stack([res.results[b]["out"] for b in range(B)], axis=0)


# revision 34
# speedup vs baseline: 1.1320x; 1.1320x over previous
"""TRN2 Bass kernel for nn_Attention_70257075028315.

reference:
    scores = einsum('bqd,bkd->bqk', query, key)       # B=8, Nq=Nk=2048, D=512
    probs  = softmax(scores, -1)
    out    = einsum('bqk,bkd->bqd', probs, key)

Sharding: batch b -> NeuronCore b (data parallel, fully local attention).

v3 design (per-core q/k: [2048, 512] fp16 -- host casts fp32->fp16 so the
device never sees fp32 inputs; halves HBM load traffic and kills the cast):

  Load (all DMA, zero PE/DVE/ACT work):
    k16 [P, 16t, 512]  natural fp16, plain SWDGE loads (gpsimd queue)
    kT  [P, 4g, 4dc, 4t, P] via direct DRAM->SBUF XBAR dma-transposes, one
        per 4-tile group ([512,512] fp16 source). Group-major layout makes
        the S rhs slice for score-chunk c (= K group c) contiguous.
    qT  [P, 4g, 4dc, 4t, P] same, lazily per group (g0 up front).
    16 warm-up matmuls on a zero tile run during the DMA fill so the PE's
    HAM clock-gate (1.2->2.4 GHz after ~3.4us busy) is warm for S(0).

  Phase C per q-tile i (steady state, software-pipelined, lookahead=2):
    S     16 fp16 matmuls (4 dc-accum x 4 chunks) in PAIR-interleaved order
          (c0,c1 alternate, then c2,c3): consecutive MMs hit different PSUM
          banks so accumulation RMW chains never serialize (same-bank
          back-to-back accum MMs cost the full ~379ns isolated-MM latency
          instead of ~131ns streamed -- HW-measured), while chunk c0/c1
          still finish at MM#7/8 so the max pipeline starts at mid-tile.
    max   per-chunk DVE row-max emitted at each chunk's stop (DVE reduce is
          1x-mode, one PSUM operand per instruction), then negate-combine;
          pair order hides the c0/c1 maxes under S's second half.
    p     = exp(S - max) fp16, one ACT pass per chunk with fused row-sum
          accum; chunk-wise XBAR SBUF->SBUF transposes into pT as each
          exp lands (xbar_splits=4; 1 and 2 measured slower).
    PV    16 fp16 matmuls, accumulation INTERLEAVED over 2 PSUM banks
          (t%2) so the 16-deep accumulation chain becomes 2 chains and
          streams at MM rate; partials merged on DVE (copy + one
          PSUM+SBUF add -- the "fused" ACT-PSUM-mul variant measured
          1.5us/tile slower) and scaled by 1/rowsum on ACT, out-DMA on
          the Pool queue. Row-sum combine + reciprocal are emitted after
          the merge adds so the DVE queue never idles on fresh exps.
  PSUM: 6 "s" banks (ring; S(i+1) never waits on tile i's exp chain) +
  2 "pv" banks.

Measured: v2 baseline phase C 143.2us (load +74us on top); v3 phase C
118.9us, rel err 1.505e-03. Decomposition that motivated v3: S-only
31.3us/16 tiles (fp16 MMs stream ~2 cols/cycle, ~122ns per N=512 MM);
S+PV mm-only 135.6us -> v2 PV's single-bank accumulation chain ran at
the ~379ns isolated-MM latency per MM. lookahead=3, xbar_splits in
{1,2}, s_order dc/c, and ACT-side merges all measured worse.
"""

import numpy as np

import concourse.bass as bass
import concourse.tile as tile
import concourse.mybir as mybir
from concourse import bacc
from concourse.bass_utils import run_bass_kernel_spmd

FP32 = mybir.dt.float32
FP32R = mybir.dt.float32r
FP16 = mybir.dt.float16
AF = mybir.ActivationFunctionType
ALU = mybir.AluOpType

B, NQ, NK, D = 8, 2048, 2048, 512
P = 128
NKT = NK // P   # 16 kk tiles
NQT = NQ // P   # 16 q tiles
NDC = D // P    # 4 d chunks
NCH = NK // 512  # 4 score chunks of 512
NG = 4          # 4-tile load groups


def build(score_dtype=FP16, repeat_c=1, timed=False, pv_dtype=FP16,
          xbar_splits=4, lookahead=2, s_order="pair", probe=None,
          dedup_ldw=True, timed_scope="c", pv_banks=2, warmup=16,
          merge_eng="dve", pt_queues=1):
    """timed=True adds an int32 [1,1] input "reps": phase C (timed_scope="c")
    or the whole kernel (="full") re-runs in a dynamic For_i loop `reps`
    more times, so one NEFF measures its own steady-state slope."""
    nc = bacc.Bacc("TRN2", target_bir_lowering=False, debug=False)
    q_d = nc.dram_tensor("query", [NQ, D], FP16, kind="ExternalInput").ap()
    k_d = nc.dram_tensor("key", [NK, D], FP16, kind="ExternalInput").ap()
    reps_d = None
    if timed:
        reps_d = nc.dram_tensor(
            "reps", [1, 1], mybir.dt.int32, kind="ExternalInput"
        ).ap()
    out_d = nc.dram_tensor("out", [NQ, D], FP32, kind="ExternalOutput").ap()

    with tile.TileContext(nc) as tc:
        _body(tc, q_d, k_d, out_d, score_dtype, repeat_c, reps_d, pv_dtype,
              xbar_splits, lookahead, s_order, probe, timed_scope, pv_banks,
              warmup, merge_eng, pt_queues)
    nc.compile()
    if dedup_ldw:
        # NOTE: do NOT reorder instructions here: compile() lowers waits to
        # per-engine completion-count semaphores, so same-engine reordering
        # breaks wait semantics. Removing a non-updating instruction and
        # merging its waits forward is the only safe post-compile transform.
        _dedup_ldweights(nc)
    return nc


def _dedup_ldweights(nc):
    """Drop InstLdweights that reload the exact weights already in the PE
    array (pair-order S keeps lhsT constant across 2 consecutive matmuls,
    warm-up MMs across 16). Safe: the weights value is identical, and the
    removed instruction's dependency edges are merged into its matmul."""
    def wkey(ldw):
        ap = ldw.ins[0]
        return (str(ap.memref), ap.offset, str(ap.ap), str(ap.dtype),
                str(ldw.is_transpose), str(getattr(ldw, "perf_mode", None)))

    removed = {}
    for fn in nc.m.functions:
        for blk in fn.blocks:
            insl = list(blk.instructions)
            next_pe = [None] * len(insl)
            nxt = None
            for i in range(len(insl) - 1, -1, -1):
                next_pe[i] = nxt
                if str(insl[i].engine) == "EngineType.PE":
                    nxt = insl[i]
            keep, last = [], None
            for i, ins in enumerate(insl):
                if str(ins.engine) != "EngineType.PE":
                    keep.append(ins)
                    continue
                if type(ins).__name__ != "InstLdweights":
                    keep.append(ins)
                    continue
                k = wkey(ins)
                mm = next_pe[i]
                if (k == last and mm is not None
                        and type(mm).__name__ == "InstMatmult"):
                    mm.merge_dependencies_from(ins)
                    removed[ins.name] = mm.name
                    continue
                last = k
                keep.append(ins)
            if len(keep) != len(insl):
                blk.instructions = keep
    if removed:
        for fn in nc.m.functions:
            for blk in fn.blocks:
                for ins in blk.instructions:
                    ins.remap_dependency_names(removed)


def _body(tc, q_d, k_d, out_d, score_dtype, repeat_c, reps_d, pv_dtype,
          xbar_splits, lookahead, s_order, probe, timed_scope, pv_banks,
          warmup, merge_eng="dve", pt_queues=1):
    from contextlib import ExitStack

    nc = tc.nc
    q_tiles_d = q_d.rearrange("(t p) d -> t p d", p=P)
    k_tiles_d = k_d.rearrange("(t p) d -> t p d", p=P)
    out_tiles_d = out_d.rearrange("(t p) d -> t p d", p=P)

    reps_rv = None
    if reps_d is not None:
        regs = nc.alloc_registers("reps_regs")
        nc.regs_load(regs, reps_d[0:1, 0:1])
        reps_rv = nc.snap(regs, donate=True, min_val=0, max_val=64)
    with ExitStack() as ctx:
        persist = ctx.enter_context(tc.tile_pool(name="persist", bufs=1))
        work = ctx.enter_context(tc.tile_pool(name="work", bufs=4))
        small = ctx.enter_context(tc.tile_pool(name="small", bufs=4))
        ps_s = ctx.enter_context(
            tc.tile_pool(name="ps_s", bufs=8 - pv_banks, space="PSUM"))
        ps_pv = ctx.enter_context(
            tc.tile_pool(name="ps_pv", bufs=pv_banks, space="PSUM"))

        # SBUF layouts. kT/qT group-major: one DRAM->SBUF XBAR transpose of
        # a [512, 512] fp16 source fills kT[:, g] with [d%128, dc, t, p]
        # (the XBAR writes col-block j=dc to free offset j*512 + row), so
        # the S rhs for chunk c / dchunk dc is the contiguous kT[:, c, dc].
        k16 = persist.tile([P, NKT, D], pv_dtype)
        kT = persist.tile([P, NG, NDC, 4, P], score_dtype)
        qT = persist.tile([P, NG, NDC, 4, P], score_dtype)
        qT_s = lambda dc, i: qT[:, i // 4, dc, i % 4, :]

        # Warm the ACT exp table at t=0 so the first real exp doesn't pay
        # the ~1.3us table load.
        warm = persist.tile([P, 1], FP32)
        nc.vector.memset(warm[:], 0.0)
        nc.scalar.activation(warm[:], warm[:], AF.Exp, bias=warm[:])

        # Ablation-probe scaffolding (timing only; output wrong):
        #   no_max: constant exp bias (removes DVE maxes + S->exp coupling)
        #   no_pT : constant PV lhsT (removes pT xbars + exp->PV coupling);
        #           p is DMA-consumed so walrus can't dead-code the chain.
        cbias = None
        dummy_pT = None
        sink_d = None
        if probe == "no_max":
            cbias = persist.tile([P, 1], FP32)
            nc.vector.memset(cbias[:], -60.0)
        if probe == "no_pT":
            dummy_pT = persist.tile([P, NKT, P], pv_dtype)
            nc.vector.memset(dummy_pT[:], 0.001)
            sink_d = nc.dram_tensor(
                "probe_sink", [P, NCH], pv_dtype, kind="Internal").ap()

        # PE warm-up during the DMA fill: ~16 N=512 matmuls on a zero tile
        # (~3.4us) flip the HAM clock-gate to 2.4 GHz before S(0) lands.
        if warmup:
            warm16 = persist.tile([P, 512], score_dtype)
            nc.vector.memset(warm16[:], 0.0)
            wps = ps_pv.tile([P, 512], FP32, tag="pv", name="warmup_ps")
            for _ in range(warmup):
                nc.tensor.matmul(wps[:], lhsT=warm16[:, 0:P], rhs=warm16[:],
                                 start=True, stop=True)

        def emit_kT_g(g, eng):
            eng.dma_start_transpose(
                kT[:, g].rearrange("p a t r -> p a (t r)"),
                k_d[g * 512:(g + 1) * 512, :])

        def emit_qT_g(g, eng):
            eng.dma_start_transpose(
                qT[:, g].rearrange("p a t r -> p a (t r)"),
                q_d[g * 512:(g + 1) * 512, :])

        def emit_k16_g(g, eng):
            src = k_tiles_d[g * 4:(g + 1) * 4].rearrange("t p d -> p t d")
            eng.dma_start(k16[:, g * 4:(g + 1) * 4], src)

        q_groups_emitted = [False] * NG

        def emit_head():
            q_groups_emitted[:] = [False] * NG
            # Need-order: tile 0 runs chunk-order, chunk c needs kT group c.
            # sync gets g0/g2, scalar qT g0 then g1/g3; k16 on gpsimd.
            emit_kT_g(0, nc.sync)
            emit_qT_g(0, nc.scalar)
            emit_k16_g(0, nc.gpsimd)
            emit_kT_g(1, nc.scalar)
            emit_kT_g(2, nc.sync)
            emit_kT_g(3, nc.scalar)
            for g in range(1, NG):
                emit_k16_g(g, nc.gpsimd)
            q_groups_emitted[0] = True

        def ensure_q_group(i):
            g = i // 4
            if not q_groups_emitted[g]:
                emit_qT_g(g, nc.scalar)
                q_groups_emitted[g] = True

        emit_head()

        # ---- Phase C ----
        def emit_S(i, c_order=False):
            """S matmuls + fused pairwise chunk maxes.
            Pair order: (c0,c1) alternate across dc, then (c2,c3): no
            same-bank back-to-back accumulation (see module docstring),
            chunks still complete in two waves for early max/exp start."""
            chunks = [ps_s.tile([P, 512], FP32, tag="s", name=f"s{i}_{c}")
                      for c in range(NCH)]
            m4 = small.tile([P, NCH], FP32, tag="m4")
            negmax = small.tile([P, 1], FP32, tag="negmax")
            if c_order or s_order == "c":
                loop = [(dc, c) for c in range(NCH) for dc in range(NDC)]
            elif s_order == "dc":
                loop = [(dc, c) for dc in range(NDC) for c in range(NCH)]
            else:  # pair
                loop = [(dc, cp + c) for cp in (0, 2)
                        for dc in range(NDC) for c in (0, 1)]
            last_mm = None
            for dc, c in loop:
                last_mm = nc.tensor.matmul(
                    chunks[c][:],
                    lhsT=qT_s(dc, i),
                    rhs=kT[:, c, dc],
                    start=(dc == 0),
                    stop=(dc == NDC - 1),
                )
                if dc == NDC - 1 and probe != "no_max":
                    # DVE per-chunk row-max right at the chunk's stop; pair
                    # order stops c0/c1 mid-tile so their maxes hide under
                    # S's second half. (A dual-bank [P,2,512] variant with 2
                    # FD=1024 reduces measured +60us -- do not re-try.)
                    nc.vector.reduce_max(
                        m4[:, c:c + 1], chunks[c][:],
                        axis=mybir.AxisListType.X,
                    )
            if probe == "no_max":
                return chunks, cbias, last_mm
            nc.vector.reduce_max(
                negmax[:], m4[:], axis=mybir.AxisListType.X, negate=True)
            return chunks, negmax, last_mm

        def emit_E(i, chunks, negmax):
            """exp(S - max) per chunk -> p (fp16) + fused row-sums; each
            chunk XBAR-transposed into pT as its exp lands. Row-sum
            combine + reciprocal are emitted LATER (emit_sums) so the DVE
            queue isn't blocked waiting on the last exp before running the
            PV merge adds of the tile 2 steps behind."""
            p = work.tile([P, NCH, 512], pv_dtype, tag="p")
            pT = work.tile([P, NKT, P], pv_dtype, tag="pT")
            rs4 = small.tile([P, NCH], FP32, tag="rs4")
            rowsum = small.tile([P, 1], FP32, tag="rowsum")
            rinv = small.tile([P, 1], FP32, tag="rinv")
            w = NCH // xbar_splits
            for c in range(NCH):
                nc.scalar.activation(
                    p[:, c, :], chunks[c][:], AF.Exp, bias=negmax[:],
                    accum_out=rs4[:, c:c + 1],
                )
                if probe == "no_pT":
                    continue
                if (c + 1) % w == 0:
                    s = c + 1 - w
                    xq = (nc.sync if pt_queues == 1 or i % 2 == 0
                          else nc.scalar)
                    xq.dma_start_transpose(
                        pT[:, s * 4:(c + 1) * 4, :], p[:, s:c + 1, :]
                    )
            if probe == "no_pT":
                nc.scalar.dma_start(sink_d[:, :], p[:, :, 0:1])
                return dummy_pT, (rs4, rowsum, rinv)
            return pT, (rs4, rowsum, rinv)

        def emit_sums(sums):
            rs4, rowsum, rinv = sums
            nc.vector.reduce_sum(rowsum[:], rs4[:], axis=mybir.AxisListType.X)
            nc.vector.reciprocal(rinv[:], rowsum[:])

        def emit_PV(i, pT, rinv, after=None, sums_cur=None):
            """PV with accumulation interleaved over pv_banks PSUM banks
            (t % pv_banks) so accumulation chains stream at MM rate; the
            partials are merged on DVE (+GPSIMD for the SBUF-only final
            add) and scaled by 1/rowsum on ACT."""
            nb = pv_banks
            pvs = [ps_pv.tile([P, 512], FP32, tag="pv", name=f"pv{i}_{j}")
                   for j in range(nb)]
            for t in range(NKT):
                mm = nc.tensor.matmul(
                    pvs[t % nb][:],
                    lhsT=pT[:, t, :],
                    rhs=k16[:, t, :],
                    start=(t < nb),
                    stop=(t >= NKT - nb),
                )
                if t == 0 and after is not None:
                    # Keep PV(i) behind S(i+2) on the PE queue so PV's work
                    # covers tile i+2's max->exp->xbar latency chain.
                    tile.add_dep_helper(
                        mm.ins, after.ins, False, "pv-after-next-S"
                    )
            # Merge the partials: DVE may read only one PSUM operand per
            # instruction, so copy pv0 to SBUF then add pv1 (PSUM) + copy
            # (SBUF). Scale by 1/rowsum on ACT (frees DVE for next tile's
            # maxes); row-sum combine + reciprocal are slotted here so the
            # DVE queue never waits on the freshly-emitted exps.
            # Merge + scale (A/B-measured: DVE copy + DVE add + ACT scale
            # beats the "fused" ACT-PSUM-mul variant by ~1.5us/tile).
            acc = work.tile([P, 512], FP32, tag="acc")
            merged = work.tile([P, 512], FP32, tag="merged")
            out_sb = work.tile([P, 512], FP32, tag="out_sb")
            nc.vector.tensor_copy(acc[:], pvs[0][:])
            for j in range(1, nb):
                dst = merged if j == nb - 1 else acc
                nc.vector.scalar_tensor_tensor(
                    dst[:], pvs[j][:], 0.0, acc[:], ALU.add, ALU.add)
            if sums_cur is not None:
                emit_sums(sums_cur)
            nc.scalar.mul(out_sb[:], merged[:], rinv[:])
            oq = nc.sync if i >= NQT - 1 else nc.gpsimd
            oq.dma_start(out_tiles_d[i], out_sb[:])
            return mm

        def emit_C():
            if probe == "s_only":
                for i in range(NQT):
                    ensure_q_group(i)
                    emit_S(i, c_order=(i == 0))
                return
            if probe == "mm_only":
                # Pure-PE probe: S + interleaved-bank PV with a constant
                # lhsT (no softmax coupling). Output numerically wrong.
                for i in range(NQT):
                    ensure_q_group(i)
                    emit_S(i, c_order=(i == 0))
                    nb = pv_banks
                    pvs = [ps_pv.tile([P, 512], FP32, tag="pv",
                                      name=f"mpv{i}_{j}") for j in range(nb)]
                    for t in range(NKT):
                        nc.tensor.matmul(
                            pvs[t % nb][:], lhsT=qT_s(0, i), rhs=k16[:, t, :],
                            start=(t < nb), stop=(t >= NKT - nb),
                        )
                return
            la = lookahead
            state = {}
            for j in range(la):
                ensure_q_group(j)
                s_ps, s_nm, _ = emit_S(j, c_order=(j == 0))
                pT_j, sums_j = emit_E(j, s_ps, s_nm)
                emit_sums(sums_j)
                state[j] = (pT_j, sums_j[2])
            for i in range(NQT):
                pT_i, rinv_i = state.pop(i)
                after = None
                sums_cur = None
                if i + la < NQT:
                    ensure_q_group(i + la)
                    s_ps, s_nm, after = emit_S(i + la)
                    pT_n, sums_cur = emit_E(i + la, s_ps, s_nm)
                    state[i + la] = (pT_n, sums_cur[2])
                emit_PV(i, pT_i, rinv_i, after=after, sums_cur=sums_cur)

        emit_C()
        for _ in range(repeat_c - 1):
            emit_C()

        if reps_rv is not None:
            with tc.For_i(0, reps_rv, 1):
                if timed_scope in ("full", "load"):
                    emit_head()
                if timed_scope == "load":
                    for g in range(NG):
                        ensure_q_group(g * 4)
                else:
                    emit_C()


_NC_CACHE = {}


def _get_nc(repeat_c=1):
    key = repeat_c
    if key not in _NC_CACHE:
        _NC_CACHE[key] = build(repeat_c=repeat_c)
    return _NC_CACHE[key]


def kernel(query: np.ndarray, key: np.ndarray) -> np.ndarray:
    query = np.asarray(query, dtype=np.float32)
    key = np.asarray(key, dtype=np.float32)
    assert query.shape == (B, NQ, D) and key.shape == (B, NK, D)
    q16 = query.astype(np.float16)
    k16 = key.astype(np.float16)
    nc = _get_nc()
    in_maps = [{"query": q16[b], "key": k16[b]} for b in range(B)]
    res = run_bass_kernel_spmd(nc, in_maps, list(range(B)))
    return np.stack([res.results[b]["out"] for b in range(B)], axis=0)


# revision 35
# speedup vs baseline: 1.1404x; 1.0074x over previous
"""TRN2 Bass kernel for nn_Attention_70257075028315.

reference:
    scores = einsum('bqd,bkd->bqk', query, key)       # B=8, Nq=Nk=2048, D=512
    probs  = softmax(scores, -1)
    out    = einsum('bqk,bkd->bqd', probs, key)

Sharding: batch b -> NeuronCore b (data parallel, fully local attention).

v3 design (per-core q/k: [2048, 512] fp16 -- host casts fp32->fp16 so the
device never sees fp32 inputs; halves HBM load traffic and kills the cast):

  Load (all DMA, zero PE/DVE/ACT work):
    k16 [P, 16t, 512]  natural fp16, plain SWDGE loads (gpsimd queue)
    kT  [P, 4g, 4dc, 4t, P] via direct DRAM->SBUF XBAR dma-transposes, one
        per 4-tile group ([512,512] fp16 source). Group-major layout makes
        the S rhs slice for score-chunk c (= K group c) contiguous.
    qT  [P, 4g, 4dc, 4t, P] same, lazily per group (g0 up front).
    16 warm-up matmuls on a zero tile run during the DMA fill so the PE's
    HAM clock-gate (1.2->2.4 GHz after ~3.4us busy) is warm for S(0).

  Phase C per q-tile i (steady state, software-pipelined, lookahead=2):
    S     16 fp16 matmuls (4 dc-accum x 4 chunks) in PAIR-interleaved order
          (c0,c1 alternate, then c2,c3): consecutive MMs hit different PSUM
          banks so accumulation RMW chains never serialize (same-bank
          back-to-back accum MMs cost the full ~379ns isolated-MM latency
          instead of ~131ns streamed -- HW-measured), while chunk c0/c1
          still finish at MM#7/8 so the max pipeline starts at mid-tile.
    max   per-chunk DVE row-max emitted at each chunk's stop (DVE reduce is
          1x-mode, one PSUM operand per instruction), then negate-combine;
          pair order hides the c0/c1 maxes under S's second half.
    p     = exp(S - max) fp16, one ACT pass per chunk with fused row-sum
          accum; chunk-wise XBAR SBUF->SBUF transposes into pT as each
          exp lands (xbar_splits=4; 1 and 2 measured slower).
    PV    16 fp16 matmuls, accumulation INTERLEAVED over 2 PSUM banks
          (t%2) so the 16-deep accumulation chain becomes 2 chains and
          streams at MM rate; partials merged on DVE (copy + one
          PSUM+SBUF add -- the "fused" ACT-PSUM-mul variant measured
          1.5us/tile slower) and scaled by 1/rowsum on ACT, out-DMA on
          the Pool queue. Row-sum combine + reciprocal are emitted after
          the merge adds so the DVE queue never idles on fresh exps.
  PSUM: 6 "s" banks (ring; S(i+1) never waits on tile i's exp chain) +
  2 "pv" banks.

Measured: v2 baseline phase C 143.2us (load +74us on top); v3 phase C
118.9us, rel err 1.505e-03. Decomposition that motivated v3: S-only
31.3us/16 tiles (fp16 MMs stream ~2 cols/cycle, ~122ns per N=512 MM);
S+PV mm-only 135.6us -> v2 PV's single-bank accumulation chain ran at
the ~379ns isolated-MM latency per MM. lookahead=3, xbar_splits in
{1,2}, s_order dc/c, ACT-side merges, dual-bank max reduces, and pT
transposes alternating onto the ACT HWDGE queue (pt_queues=2) ALL
measured worse (+20..+66us) -- the schedule sits in a narrow groove;
change one thing at a time and re-measure.
"""

import numpy as np

import concourse.bass as bass
import concourse.tile as tile
import concourse.mybir as mybir
from concourse import bacc
from concourse.bass_utils import run_bass_kernel_spmd

FP32 = mybir.dt.float32
FP32R = mybir.dt.float32r
FP16 = mybir.dt.float16
AF = mybir.ActivationFunctionType
ALU = mybir.AluOpType

B, NQ, NK, D = 8, 2048, 2048, 512
P = 128
NKT = NK // P   # 16 kk tiles
NQT = NQ // P   # 16 q tiles
NDC = D // P    # 4 d chunks
NCH = NK // 512  # 4 score chunks of 512
NG = 4          # 4-tile load groups


def build(score_dtype=FP16, repeat_c=1, timed=False, pv_dtype=FP16,
          xbar_splits=4, lookahead=2, s_order="pair", probe=None,
          dedup_ldw=True, timed_scope="c", pv_banks=2, warmup=16,
          merge_eng="dve", pt_queues=1):
    """timed=True adds an int32 [1,1] input "reps": phase C (timed_scope="c")
    or the whole kernel (="full") re-runs in a dynamic For_i loop `reps`
    more times, so one NEFF measures its own steady-state slope."""
    nc = bacc.Bacc("TRN2", target_bir_lowering=False, debug=False)
    q_d = nc.dram_tensor("query", [NQ, D], FP16, kind="ExternalInput").ap()
    k_d = nc.dram_tensor("key", [NK, D], FP16, kind="ExternalInput").ap()
    reps_d = None
    if timed:
        reps_d = nc.dram_tensor(
            "reps", [1, 1], mybir.dt.int32, kind="ExternalInput"
        ).ap()
    out_d = nc.dram_tensor("out", [NQ, D], FP32, kind="ExternalOutput").ap()

    with tile.TileContext(nc) as tc:
        _body(tc, q_d, k_d, out_d, score_dtype, repeat_c, reps_d, pv_dtype,
              xbar_splits, lookahead, s_order, probe, timed_scope, pv_banks,
              warmup, merge_eng, pt_queues)
    nc.compile()
    if dedup_ldw:
        # NOTE: do NOT reorder instructions here: compile() lowers waits to
        # per-engine completion-count semaphores, so same-engine reordering
        # breaks wait semantics. Removing a non-updating instruction and
        # merging its waits forward is the only safe post-compile transform.
        _dedup_ldweights(nc)
    return nc


def _dedup_ldweights(nc):
    """Drop InstLdweights that reload the exact weights already in the PE
    array (pair-order S keeps lhsT constant across 2 consecutive matmuls,
    warm-up MMs across 16). Safe: the weights value is identical, and the
    removed instruction's dependency edges are merged into its matmul."""
    def wkey(ldw):
        ap = ldw.ins[0]
        return (str(ap.memref), ap.offset, str(ap.ap), str(ap.dtype),
                str(ldw.is_transpose), str(getattr(ldw, "perf_mode", None)))

    removed = {}
    for fn in nc.m.functions:
        for blk in fn.blocks:
            insl = list(blk.instructions)
            next_pe = [None] * len(insl)
            nxt = None
            for i in range(len(insl) - 1, -1, -1):
                next_pe[i] = nxt
                if str(insl[i].engine) == "EngineType.PE":
                    nxt = insl[i]
            keep, last = [], None
            for i, ins in enumerate(insl):
                if str(ins.engine) != "EngineType.PE":
                    keep.append(ins)
                    continue
                if type(ins).__name__ != "InstLdweights":
                    keep.append(ins)
                    continue
                k = wkey(ins)
                mm = next_pe[i]
                if (k == last and mm is not None
                        and type(mm).__name__ == "InstMatmult"):
                    mm.merge_dependencies_from(ins)
                    removed[ins.name] = mm.name
                    continue
                last = k
                keep.append(ins)
            if len(keep) != len(insl):
                blk.instructions = keep
    if removed:
        for fn in nc.m.functions:
            for blk in fn.blocks:
                for ins in blk.instructions:
                    ins.remap_dependency_names(removed)


def _body(tc, q_d, k_d, out_d, score_dtype, repeat_c, reps_d, pv_dtype,
          xbar_splits, lookahead, s_order, probe, timed_scope, pv_banks,
          warmup, merge_eng="dve", pt_queues=1):
    from contextlib import ExitStack

    nc = tc.nc
    q_tiles_d = q_d.rearrange("(t p) d -> t p d", p=P)
    k_tiles_d = k_d.rearrange("(t p) d -> t p d", p=P)
    out_tiles_d = out_d.rearrange("(t p) d -> t p d", p=P)

    reps_rv = None
    if reps_d is not None:
        regs = nc.alloc_registers("reps_regs")
        nc.regs_load(regs, reps_d[0:1, 0:1])
        reps_rv = nc.snap(regs, donate=True, min_val=0, max_val=64)
    with ExitStack() as ctx:
        persist = ctx.enter_context(tc.tile_pool(name="persist", bufs=1))
        work = ctx.enter_context(tc.tile_pool(name="work", bufs=4))
        small = ctx.enter_context(tc.tile_pool(name="small", bufs=4))
        ps_s = ctx.enter_context(
            tc.tile_pool(name="ps_s", bufs=8 - pv_banks, space="PSUM"))
        ps_pv = ctx.enter_context(
            tc.tile_pool(name="ps_pv", bufs=pv_banks, space="PSUM"))

        # SBUF layouts. kT/qT group-major: one DRAM->SBUF XBAR transpose of
        # a [512, 512] fp16 source fills kT[:, g] with [d%128, dc, t, p]
        # (the XBAR writes col-block j=dc to free offset j*512 + row), so
        # the S rhs for chunk c / dchunk dc is the contiguous kT[:, c, dc].
        k16 = persist.tile([P, NKT, D], pv_dtype)
        kT = persist.tile([P, NG, NDC, 4, P], score_dtype)
        qT = persist.tile([P, NG, NDC, 4, P], score_dtype)
        qT_s = lambda dc, i: qT[:, i // 4, dc, i % 4, :]

        # Warm the ACT exp table at t=0 so the first real exp doesn't pay
        # the ~1.3us table load.
        warm = persist.tile([P, 1], FP32)
        nc.vector.memset(warm[:], 0.0)
        nc.scalar.activation(warm[:], warm[:], AF.Exp, bias=warm[:])

        # Ablation-probe scaffolding (timing only; output wrong):
        #   no_max: constant exp bias (removes DVE maxes + S->exp coupling)
        #   no_pT : constant PV lhsT (removes pT xbars + exp->PV coupling);
        #           p is DMA-consumed so walrus can't dead-code the chain.
        cbias = None
        dummy_pT = None
        sink_d = None
        if probe == "no_max":
            cbias = persist.tile([P, 1], FP32)
            nc.vector.memset(cbias[:], -60.0)
        if probe == "no_pT":
            dummy_pT = persist.tile([P, NKT, P], pv_dtype)
            nc.vector.memset(dummy_pT[:], 0.001)
            sink_d = nc.dram_tensor(
                "probe_sink", [P, NCH], pv_dtype, kind="Internal").ap()

        # PE warm-up during the DMA fill: ~16 N=512 matmuls on a zero tile
        # (~3.4us) flip the HAM clock-gate to 2.4 GHz before S(0) lands.
        if warmup:
            warm16 = persist.tile([P, 512], score_dtype)
            nc.vector.memset(warm16[:], 0.0)
            wps = ps_pv.tile([P, 512], FP32, tag="pv", name="warmup_ps")
            for _ in range(warmup):
                nc.tensor.matmul(wps[:], lhsT=warm16[:, 0:P], rhs=warm16[:],
                                 start=True, stop=True)

        def emit_kT_g(g, eng):
            eng.dma_start_transpose(
                kT[:, g].rearrange("p a t r -> p a (t r)"),
                k_d[g * 512:(g + 1) * 512, :])

        def emit_qT_g(g, eng):
            eng.dma_start_transpose(
                qT[:, g].rearrange("p a t r -> p a (t r)"),
                q_d[g * 512:(g + 1) * 512, :])

        def emit_k16_g(g, eng):
            src = k_tiles_d[g * 4:(g + 1) * 4].rearrange("t p d -> p t d")
            eng.dma_start(k16[:, g * 4:(g + 1) * 4], src)

        q_groups_emitted = [False] * NG

        def emit_head():
            q_groups_emitted[:] = [False] * NG
            # Need-order: tile 0 runs chunk-order, chunk c needs kT group c.
            # sync gets g0/g2, scalar qT g0 then g1/g3; k16 on gpsimd.
            emit_kT_g(0, nc.sync)
            emit_qT_g(0, nc.scalar)
            emit_k16_g(0, nc.gpsimd)
            emit_kT_g(1, nc.scalar)
            emit_kT_g(2, nc.sync)
            emit_kT_g(3, nc.scalar)
            for g in range(1, NG):
                emit_k16_g(g, nc.gpsimd)
            q_groups_emitted[0] = True

        def ensure_q_group(i):
            g = i // 4
            if not q_groups_emitted[g]:
                emit_qT_g(g, nc.scalar)
                q_groups_emitted[g] = True

        emit_head()

        # ---- Phase C ----
        def emit_S(i, c_order=False):
            """S matmuls + fused pairwise chunk maxes.
            Pair order: (c0,c1) alternate across dc, then (c2,c3): no
            same-bank back-to-back accumulation (see module docstring),
            chunks still complete in two waves for early max/exp start."""
            chunks = [ps_s.tile([P, 512], FP32, tag="s", name=f"s{i}_{c}")
                      for c in range(NCH)]
            m4 = small.tile([P, NCH], FP32, tag="m4")
            negmax = small.tile([P, 1], FP32, tag="negmax")
            if c_order or s_order == "c":
                loop = [(dc, c) for c in range(NCH) for dc in range(NDC)]
            elif s_order == "dc":
                loop = [(dc, c) for dc in range(NDC) for c in range(NCH)]
            else:  # pair
                loop = [(dc, cp + c) for cp in (0, 2)
                        for dc in range(NDC) for c in (0, 1)]
            last_mm = None
            for dc, c in loop:
                last_mm = nc.tensor.matmul(
                    chunks[c][:],
                    lhsT=qT_s(dc, i),
                    rhs=kT[:, c, dc],
                    start=(dc == 0),
                    stop=(dc == NDC - 1),
                )
                if dc == NDC - 1 and probe != "no_max":
                    # DVE per-chunk row-max right at the chunk's stop; pair
                    # order stops c0/c1 mid-tile so their maxes hide under
                    # S's second half. (A dual-bank [P,2,512] variant with 2
                    # FD=1024 reduces measured +60us -- do not re-try.)
                    nc.vector.reduce_max(
                        m4[:, c:c + 1], chunks[c][:],
                        axis=mybir.AxisListType.X,
                    )
            if probe == "no_max":
                return chunks, cbias, last_mm
            nc.vector.reduce_max(
                negmax[:], m4[:], axis=mybir.AxisListType.X, negate=True)
            return chunks, negmax, last_mm

        def emit_E(i, chunks, negmax):
            """exp(S - max) per chunk -> p (fp16) + fused row-sums; each
            chunk XBAR-transposed into pT as its exp lands. Row-sum
            combine + reciprocal are emitted LATER (emit_sums) so the DVE
            queue isn't blocked waiting on the last exp before running the
            PV merge adds of the tile 2 steps behind."""
            p = work.tile([P, NCH, 512], pv_dtype, tag="p")
            pT = work.tile([P, NKT, P], pv_dtype, tag="pT")
            rs4 = small.tile([P, NCH], FP32, tag="rs4")
            rowsum = small.tile([P, 1], FP32, tag="rowsum")
            rinv = small.tile([P, 1], FP32, tag="rinv")
            w = NCH // xbar_splits
            for c in range(NCH):
                nc.scalar.activation(
                    p[:, c, :], chunks[c][:], AF.Exp, bias=negmax[:],
                    accum_out=rs4[:, c:c + 1],
                )
                if probe == "no_pT":
                    continue
                if (c + 1) % w == 0:
                    s = c + 1 - w
                    xq = (nc.sync if pt_queues == 1 or i % 2 == 0
                          else nc.scalar)
                    xq.dma_start_transpose(
                        pT[:, s * 4:(c + 1) * 4, :], p[:, s:c + 1, :]
                    )
            if probe == "no_pT":
                nc.scalar.dma_start(sink_d[:, :], p[:, :, 0:1])
                return dummy_pT, (rs4, rowsum, rinv)
            return pT, (rs4, rowsum, rinv)

        def emit_sums(sums):
            rs4, rowsum, rinv = sums
            nc.vector.reduce_sum(rowsum[:], rs4[:], axis=mybir.AxisListType.X)
            nc.vector.reciprocal(rinv[:], rowsum[:])

        def emit_PV(i, pT, rinv, after=None, sums_cur=None):
            """PV with accumulation interleaved over pv_banks PSUM banks
            (t % pv_banks) so accumulation chains stream at MM rate; the
            partials are merged on DVE (+GPSIMD for the SBUF-only final
            add) and scaled by 1/rowsum on ACT."""
            nb = pv_banks
            pvs = [ps_pv.tile([P, 512], FP32, tag="pv", name=f"pv{i}_{j}")
                   for j in range(nb)]
            for t in range(NKT):
                mm = nc.tensor.matmul(
                    pvs[t % nb][:],
                    lhsT=pT[:, t, :],
                    rhs=k16[:, t, :],
                    start=(t < nb),
                    stop=(t >= NKT - nb),
                )
                if t == 0 and after is not None:
                    # Keep PV(i) behind S(i+2) on the PE queue so PV's work
                    # covers tile i+2's max->exp->xbar latency chain.
                    tile.add_dep_helper(
                        mm.ins, after.ins, False, "pv-after-next-S"
                    )
            # Merge the partials: DVE may read only one PSUM operand per
            # instruction, so copy pv0 to SBUF then add pv1 (PSUM) + copy
            # (SBUF). Scale by 1/rowsum on ACT (frees DVE for next tile's
            # maxes); row-sum combine + reciprocal are slotted here so the
            # DVE queue never waits on the freshly-emitted exps.
            # Merge + scale (A/B-measured: DVE copy + DVE add + ACT scale
            # beats the "fused" ACT-PSUM-mul variant by ~1.5us/tile).
            acc = work.tile([P, 512], FP32, tag="acc")
            merged = work.tile([P, 512], FP32, tag="merged")
            out_sb = work.tile([P, 512], FP32, tag="out_sb")
            nc.vector.tensor_copy(acc[:], pvs[0][:])
            for j in range(1, nb):
                dst = merged if j == nb - 1 else acc
                nc.vector.scalar_tensor_tensor(
                    dst[:], pvs[j][:], 0.0, acc[:], ALU.add, ALU.add)
            if sums_cur is not None:
                emit_sums(sums_cur)
            nc.scalar.mul(out_sb[:], merged[:], rinv[:])
            oq = nc.sync if i >= NQT - 1 else nc.gpsimd
            oq.dma_start(out_tiles_d[i], out_sb[:])
            return mm

        def emit_C():
            if probe == "s_only":
                for i in range(NQT):
                    ensure_q_group(i)
                    emit_S(i, c_order=(i == 0))
                return
            if probe == "mm_only":
                # Pure-PE probe: S + interleaved-bank PV with a constant
                # lhsT (no softmax coupling). Output numerically wrong.
                for i in range(NQT):
                    ensure_q_group(i)
                    emit_S(i, c_order=(i == 0))
                    nb = pv_banks
                    pvs = [ps_pv.tile([P, 512], FP32, tag="pv",
                                      name=f"mpv{i}_{j}") for j in range(nb)]
                    for t in range(NKT):
                        nc.tensor.matmul(
                            pvs[t % nb][:], lhsT=qT_s(0, i), rhs=k16[:, t, :],
                            start=(t < nb), stop=(t >= NKT - nb),
                        )
                return
            la = lookahead
            state = {}
            for j in range(la):
                ensure_q_group(j)
                s_ps, s_nm, _ = emit_S(j, c_order=(j == 0))
                pT_j, sums_j = emit_E(j, s_ps, s_nm)
                emit_sums(sums_j)
                state[j] = (pT_j, sums_j[2])
            for i in range(NQT):
                pT_i, rinv_i = state.pop(i)
                after = None
                sums_cur = None
                if i + la < NQT:
                    ensure_q_group(i + la)
                    s_ps, s_nm, after = emit_S(i + la)
                    pT_n, sums_cur = emit_E(i + la, s_ps, s_nm)
                    state[i + la] = (pT_n, sums_cur[2])
                emit_PV(i, pT_i, rinv_i, after=after, sums_cur=sums_cur)

        emit_C()
        for _ in range(repeat_c - 1):
            emit_C()

        if reps_rv is not None:
            with tc.For_i(0, reps_rv, 1):
                if timed_scope in ("full", "load"):
                    emit_head()
                if timed_scope == "load":
                    for g in range(NG):
                        ensure_q_group(g * 4)
                else:
                    emit_C()


_NC_CACHE = {}


def _get_nc(repeat_c=1):
    key = repeat_c
    if key not in _NC_CACHE:
        _NC_CACHE[key] = build(repeat_c=repeat_c)
    return _NC_CACHE[key]


def kernel(query: np.ndarray, key: np.ndarray) -> np.ndarray:
    query = np.asarray(query, dtype=np.float32)
    key = np.asarray(key, dtype=np.float32)
    assert query.shape == (B, NQ, D) and key.shape == (B, NK, D)
    q16 = query.astype(np.float16)
    k16 = key.astype(np.float16)
    nc = _get_nc()
    in_maps = [{"query": q16[b], "key": k16[b]} for b in range(B)]
    res = run_bass_kernel_spmd(nc, in_maps, list(range(B)))
    return np.stack([res.results[b]["out"] for b in range(B)], axis=0)
